# revision 26
# baseline (speedup 1.0000x reference)
"""LConv (7x7 position-linear conv) Trainium2 Bass kernel.

Full inputs in, full output out. Sharding: data-parallel over batch,
16 images -> 8 NeuronCores (2 images/core). abc/bias replicated.

Math (K=7, S=1, P=3, D=1, G=1, C=COUT=128):
  out[o,i,j] = sum_c sum_{t=1..7} P_t[c,o]*W1[c,i+t-4,j]
             + sum_c B[c,o]*box7_H(w2)[c,i,j] + bias[o]
  W1 = 7-tap box along W of padded x; w2 = centered 7-tap ramp along W.
  P_t = (t-4)*A + C ; A=abc[0:128], B=abc[128:256], C=abc[256:384].

Rows are stored 124 wide (8 lead zeros + 112 data + 4 trail) so stream
cumsums telescope exactly:
  W1[k] = (S[k+8]-S[k]) - x[k+8]        BOXW7X2 - hand 2x-packed bf16 op
  w2[k] = 7*S0 + 4*(S1-S0) - cumsum(S1-S0), S1 = cumsum(x[+7])
                                        RAMPW7 - hand 1x op
box7_H(w2) is folded into the PE accumulation via pair/quad/oct row
sums (stock bf16 tensor_tensor at 2x, emitted in per-slab chunks so the
B-taps unlock progressively): box7 = oct[i+1] - w2[i+8], so each output
tile is 9 matmuls (7 w1-taps with the merged P_t weights + 2 w2-taps,
the second against a host-negated -B weight).
"""

import numpy as np

import concourse.bacc as bacc
import concourse.mybir as mybir
from concourse import tile
from concourse.bass_utils import run_bass_kernel_spmd

F32 = mybir.dt.float32
BF16 = mybir.dt.bfloat16
AF = mybir.ActivationFunctionType
ALU = mybir.AluOpType

B_TOT = 16
N_CORES = 8
B_PER = B_TOT // N_CORES
CIN = 128
COUT = 128
H = W = 112

LEAD = 8                  # row layout: 8 lead zeros + 112 data + 4 trail
TRAIL = 4
PW = LEAD + W + TRAIL     # 124 (even -> rows stay 4B-aligned in bf16)
COL0 = 4                  # rhs data col j lives at stream col j+4
GROWS = 119               # grid rows: 3 zeros + 112 data + 4 zeros
RTOP = 3                  # grid row of x-row 0
GFULL = GROWS * PW        # 14756

ROWS_PER_SLAB = 16
N_SLABS = H // ROWS_PER_SLAB
SLAB_FREE = ROWS_PER_SLAB * PW         # 1984
XP_FREE = SLAB_FREE + 8                # 8 guard zeros for the offset reads

OUT_TILE_ROWS = 4
N_OUT_TILES = H // OUT_TILE_ROWS
OUT_TILE_FREE = OUT_TILE_ROWS * W      # 448

_CACHE = {}


class _HandDveOp:
    """Duck-typed DveOp whose uop program is hand-authored (the Spec DSL
    cannot express scan-of-scan or packed 2x programs)."""

    def __init__(self, name, spec, subdim, build):
        self.name = name
        self.spec = spec
        self.subdim = subdim
        self._build = build
        self._cache = {}

    def compile(self, ver):
        if ver not in self._cache:
            self._cache[ver] = self._build(self.name, ver)
        return self._cache[ver]


def _register(name, build, reference):
    from concourse.dve_spec import Spec, Src0, Src1, scan, AluOp
    import concourse.dve_ops as dve_ops

    for op in dve_ops.OPS:
        if op.name == name:
            return op
    spec = Spec(
        body=scan(AluOp.ADD, Src0) - scan(AluOp.ADD, Src1),
        reference=reference,
    )
    row = dve_ops._CUSTOM_DVE_ROW_BASE + len(dve_ops.OPS)
    op = _HandDveOp(name, spec, False, build)
    dve_ops.OPS.append(op)
    dve_ops._SUB_OPCODE_FOR_NAME[name] = row
    dve_ops.CUSTOM_DVE_SPECS[name] = spec
    return op


def _mk_uop(UopConfig, wires):
    u = UopConfig()
    for sel, slot in wires:
        u.enable_input(sel, slot)
    return u


def _build_rampw7_uops(name, ver):
    """out = 7*S0 + 4*D - E ; S0=scan(src0), S1=scan(src1=x[+7]),
    D=S1-S0, E=scan(D). == sum_{t=1..7} (t-4)*x[k+t] given 7+ lead zeros."""
    from concourse.dve_uop import (
        DveOpSpec, UopConfig, UopDpConfig, AluOp, AluInp, InpSel, OutSel,
        OutPath, DelayInp, Trigger,
    )
    import concourse.dve_ops as dve_ops

    PREV, CURR = AluInp.PREV_ALU_OUT, AluInp.CURR_ALU_OUT

    def lane(n):
        return AluInp(int(AluInp.PREV_DELAY_0) + n)

    wires = [
        (InpSel.SRC_0, 1), (InpSel.SRC_1, 2),
        (InpSel.CONST_0, 3), (InpSel.CONST_1, 4), (InpSel.ZERO, 5),
    ]

    def body_dp():
        dp = [UopDpConfig() for _ in range(8)]
        for st in range(8):
            dp[st].pass_through_delay(0, 1, 2, 3, 4)
        dp[0].enable_alu(AluOp.ADD, CURR, lane(0))
        dp[1].enable_alu(AluOp.ADD, CURR, lane(1))
        dp[1].enable_delay_from_src(DelayInp.PREV_ALU_OUT, 0)
        dp[2].enable_alu(AluOp.SUBTRACT, PREV, lane(0))
        dp[3].enable_alu(AluOp.ADD, CURR, PREV)
        dp[3].enable_delay_from_src(DelayInp.PREV_ALU_OUT, 1)
        dp[4].enable_alu(AluOp.MULTIPLY, lane(0), lane(2))
        dp[4].enable_delay_from_src(DelayInp.PREV_ALU_OUT, 2)
        dp[5].enable_alu(AluOp.MULTIPLY, lane(1), lane(3))
        dp[5].enable_delay_from_src(DelayInp.PREV_ALU_OUT, 3)
        dp[6].enable_alu(AluOp.ADD, PREV, lane(3))
        dp[7].enable_alu(AluOp.SUBTRACT, PREV, lane(2))
        return dp

    seed = _mk_uop(UopConfig, wires)
    seed.datapath_config = body_dp()
    for b in (0, 1, 3):
        seed.datapath_config[b].enable_alu(AluOp.BYPASS, lane(4), lane(4))
    seed.trigger = (Trigger.COUNT, Trigger.NONE, Trigger.NONE)
    seed.repeat_count = 1
    seed.next_uop = (1, 0, 0)

    steady = _mk_uop(UopConfig, wires)
    steady.datapath_config = body_dp()
    steady.trigger = (Trigger.SRC_TENSOR_DONE, Trigger.NONE, Trigger.NONE)
    steady.require_inp0 = 1
    steady.require_inp1 = 1
    steady.enable_output(OutSel.ALU_OUT, OutPath.WR0_LO)

    spec = DveOpSpec(
        name=name, opcode=dve_ops.get_dve_sub_opcode(name),
        uops=[seed, steady], rd1_en=True,
    )
    spec.validate(ver)
    return spec


def _build_boxw7_uops(name, ver):
    """out[k] = sum_{t=1..7} x[k+t] = (S[k+8]-S[k]) - x[k+8], with
    src1 = x[+8] (16B offset keeps bf16 2x mode alignable)."""
    from concourse.dve_uop import (
        DveOpSpec, UopConfig, UopDpConfig, AluOp, AluInp, InpSel, OutSel,
        OutPath, DelayInp, Trigger,
    )
    import concourse.dve_ops as dve_ops

    PREV, CURR = AluInp.PREV_ALU_OUT, AluInp.CURR_ALU_OUT

    def lane(n):
        return AluInp(int(AluInp.PREV_DELAY_0) + n)

    # ---- 1x fallback ----
    wires1 = [(InpSel.SRC_0, 1), (InpSel.SRC_1, 2), (InpSel.ZERO, 3)]

    def dp1():
        dp = [UopDpConfig() for _ in range(8)]
        for st in range(8):
            dp[st].pass_through_delay(0, 1, 2)
        dp[0].enable_alu(AluOp.ADD, CURR, lane(0))
        dp[1].enable_alu(AluOp.ADD, CURR, lane(1))
        dp[1].enable_delay_from_src(DelayInp.PREV_ALU_OUT, 0)
        dp[2].enable_alu(AluOp.SUBTRACT, PREV, lane(0))
        dp[3].enable_alu(AluOp.SUBTRACT, PREV, lane(1))
        for st in range(4, 8):
            dp[st].pass_through_alu()
        return dp

    seed1 = _mk_uop(UopConfig, wires1)
    seed1.datapath_config = dp1()
    for b in (0, 1):
        seed1.datapath_config[b].enable_alu(AluOp.BYPASS, lane(2), lane(2))
    seed1.trigger = (Trigger.COUNT, Trigger.NONE, Trigger.NONE)
    seed1.repeat_count = 1
    seed1.next_uop = (1, 0, 0)
    steady1 = _mk_uop(UopConfig, wires1)
    steady1.datapath_config = dp1()
    steady1.trigger = (Trigger.SRC_TENSOR_DONE, Trigger.NONE, Trigger.NONE)
    steady1.require_inp0 = 1
    steady1.require_inp1 = 1
    steady1.enable_output(OutSel.ALU_OUT, OutPath.WR0_LO)

    # ---- 2x packed ----
    wires2 = [
        (InpSel.SRC_0, 1), (InpSel.SRC_0_HI, 2),
        (InpSel.SRC_1, 3), (InpSel.SRC_1_HI, 4), (InpSel.ZERO, 5),
    ]

    def dp2():
        dp = [UopDpConfig() for _ in range(8)]
        for st in range(8):
            dp[st].pass_through_delay(0, 1, 2, 3, 4)
        dp[0].enable_alu(AluOp.ADD, lane(0), lane(1))      # t0 = lo0+hi0
        dp[1].enable_alu(AluOp.ADD, CURR, PREV)            # sA = S[2m+1]
        dp[2].enable_alu(AluOp.ADD, lane(2), lane(3))      # t1 = lo1+hi1
        dp[2].enable_delay_from_src(DelayInp.PREV_ALU_OUT, 0)   # sA -> l0
        dp[3].enable_alu(AluOp.ADD, CURR, PREV)            # sB = S[2m+9]
        dp[4].enable_alu(AluOp.SUBTRACT, PREV, lane(0))    # Bo = sB-sA
        dp[5].enable_alu(AluOp.SUBTRACT, PREV, lane(3))    # W1o = Bo-hi1
        dp[6].enable_alu(AluOp.ADD, PREV, lane(1))         # u = W1o+hi0
        dp[6].enable_delay_from_src(DelayInp.PREV_ALU_OUT, 3)   # W1o -> l3
        dp[7].enable_alu(AluOp.SUBTRACT, PREV, lane(2))    # W1e = u-lo1
        return dp

    seed2 = _mk_uop(UopConfig, wires2)
    seed2.datapath_config = dp2()
    for b in (1, 3):
        seed2.datapath_config[b].enable_alu(AluOp.BYPASS, lane(4), lane(4))
    seed2.trigger = (Trigger.COUNT, Trigger.NONE, Trigger.NONE)
    seed2.repeat_count = 1
    seed2.next_uop = (1, 0, 0)
    steady2 = _mk_uop(UopConfig, wires2)
    steady2.datapath_config = dp2()
    steady2.trigger = (Trigger.SRC_TENSOR_DONE, Trigger.NONE, Trigger.NONE)
    steady2.require_inp0 = 1
    steady2.require_inp1 = 1
    steady2.enable_output(OutSel.ALU_OUT, OutPath.WR0_LO)
    steady2.enable_output(OutSel.DELAY_3, OutPath.WR0_HI)

    spec = DveOpSpec(
        name=name, opcode=dve_ops.get_dve_sub_opcode(name),
        uops=[seed1, steady1], uops_2x=[seed2, steady2],
        perf_max=1, rd1_en=True,
    )
    spec.validate(ver)
    return spec


def _ramp_ref(in0, in1, *a):
    s0 = np.cumsum(in0, axis=-1, dtype=np.float32)
    s1 = np.cumsum(in1, axis=-1, dtype=np.float32)
    d = s1 - s0
    return 7.0 * s0 + 4.0 * d - np.cumsum(d, axis=-1, dtype=np.float32)


def _box_ref(in0, in1, *a):
    s0 = np.cumsum(in0, axis=-1, dtype=np.float32)
    s1 = np.cumsum(in1, axis=-1, dtype=np.float32)
    return (s1 - s0) - in1


def _build():
    nc = bacc.Bacc("TRN2", target_bir_lowering=False, debug=False)
    op_box = _register("BOXW7X2", _build_boxw7_uops, _box_ref)
    op_ramp = _register("RAMPW7", _build_rampw7_uops, _ramp_ref)

    t_x = nc.dram_tensor(
        "xs", [B_PER, CIN, H * PW + 8], BF16, kind="ExternalInput"
    )
    t_pw = nc.dram_tensor("pw", [7, CIN, COUT], BF16, kind="ExternalInput")
    t_bw = nc.dram_tensor("bw", [2, CIN, COUT], BF16, kind="ExternalInput")
    t_bias = nc.dram_tensor("bias", [COUT, 1], F32, kind="ExternalInput")
    t_out = nc.dram_tensor("out", [B_PER, COUT, H, W], BF16, kind="ExternalOutput")

    with tile.TileContext(nc) as tc:
        with (
            tc.tile_pool(name="const", bufs=1) as cpool,
            tc.tile_pool(name="wfull", bufs=1) as wpool,
            tc.tile_pool(name="slab", bufs=2) as spool,
            tc.tile_pool(name="outs", bufs=4) as opool,
            tc.tile_pool(name="ps", bufs=8, space="PSUM") as ppool,
        ):
            # first slab's input DMA goes ahead of the const DMAs so the
            # DVE filter chain (and with it the first matmul) starts ~4us
            # earlier; the Sync queue dispatches descriptors in order.
            xp_bufs = []
            for i in range(2):
                xpb = spool.tile([CIN, XP_FREE], BF16, tag=f"xp{i}")
                xp_bufs.append(xpb)
            nc.sync.dma_start(xp_bufs[0][:], t_x[0, :, 0:XP_FREE])

            # ---- constants (host-prequantized bf16) ----
            pw = cpool.tile([CIN, 7 * COUT], BF16, tag="pwb")
            nc.sync.dma_start(
                pw[:].rearrange("c (t o) -> c t o", t=7),
                t_pw[:].transpose([1, 0, 2]),
            )
            bw2 = cpool.tile([CIN, 2 * COUT], BF16, tag="bwb")
            nc.sync.dma_start(
                bw2[:].rearrange("c (s o) -> c s o", s=2),
                t_bw[:].transpose([1, 0, 2]),
            )
            bw = bw2[:, :COUT]        # +B
            bwn = bw2[:, COUT:]       # -B
            bias_sb = cpool.tile([COUT, 1], F32, tag="bias")
            nc.sync.dma_start(bias_sb[:], t_bias[:])

            # ---- streams (all row-major [GROWS x PW] grids, bf16) ----
            w1 = wpool.tile([CIN, GFULL], BF16, tag="w1")
            w2 = wpool.tile([CIN, GFULL], BF16, tag="w2")
            w2p = wpool.tile([CIN, GFULL], BF16, tag="w2p")
            w2q = wpool.tile([CIN, GFULL], BF16, tag="w2q")
            w2o = wpool.tile([CIN, GFULL], BF16, tag="w2o")
            for buf in (w1, w2):
                nc.gpsimd.memset(buf[:, : RTOP * PW], 0.0)
                nc.gpsimd.memset(buf[:, (RTOP + H) * PW :], 0.0)

            for b in range(B_PER):
                # ---------- stage 1: W-direction filters ----------
                for s in range(N_SLABS):
                    r0 = s * ROWS_PER_SLAB
                    xp = xp_bufs[s % 2]
                    if not (b == 0 and s == 0):
                        nc.sync.dma_start(
                            xp[:],
                            t_x[b, :, r0 * PW : r0 * PW + XP_FREE],
                        )
                    g0 = (RTOP + r0) * PW
                    inst = nc.vector._custom_dve(
                        op_box,
                        out=w1[:, g0 : g0 + SLAB_FREE],
                        in0=xp[:, :SLAB_FREE],
                        in1=xp[:, 8 : 8 + SLAB_FREE],
                    )
                    inst.ins.perf_max = 1

                    nc.vector._custom_dve(
                        op_ramp,
                        out=w2[:, g0 : g0 + SLAB_FREE],
                        in0=xp[:, :SLAB_FREE],
                        in1=xp[:, 7 : 7 + SLAB_FREE],
                        s0=7.0,
                        s1=4.0,
                    )

                    # H-direction pair/quad/oct rows, chunked per slab so
                    # the B-taps unblock progressively:
                    #   w2p[r] = w2[r] + w2[r+1]; w2q[r] = w2p[r] + w2p[r+2]
                    #   w2o[r] = w2q[r] + w2q[r+4]   (8-row sum)
                    def pq_chunk(eng, dst, src, lag, rlo, rhi):
                        if rhi < rlo:
                            return
                        a, bnd = rlo * PW, (rhi + 1) * PW
                        eng.tensor_tensor(
                            dst[:, a:bnd],
                            src[:, a:bnd],
                            src[:, a + lag * PW : bnd + lag * PW],
                            op=ALU.add,
                        )

                    vec = nc.vector
                    plo = 0 if s == 0 else 16 * s + 2
                    pq_chunk(vec, w2p, w2, 1, plo, 16 * s + 17)
                    qlo = 0 if s == 0 else 16 * s
                    pq_chunk(vec, w2q, w2p, 2, qlo, 16 * s + 15)
                    olo = 0 if s == 0 else 16 * s - 4
                    pq_chunk(vec, w2o, w2q, 4, olo, 16 * s + 11)
                    if s == N_SLABS - 1:
                        pq_chunk(vec, w2p, w2, 1, 16 * s + 18, GROWS - 2)
                        pq_chunk(vec, w2q, w2p, 2, 16 * s + 16, GROWS - 4)
                        pq_chunk(vec, w2o, w2q, 4, 16 * s + 12, GROWS - 8)

                # ---------- stage 2: PE taps ----------
                for it in range(N_OUT_TILES):
                    i0 = it * OUT_TILE_ROWS
                    acc = ppool.tile([COUT, OUT_TILE_FREE], F32, tag="acc")

                    def rhs(buf, trow):
                        # grid rows (i0+trow-1)..+3, data cols
                        base = (i0 + trow - 1) * PW
                        return buf[
                            :, base : base + OUT_TILE_ROWS * PW
                        ].rearrange("c (r q) -> c r q", q=PW)[
                            :, :, COL0 : COL0 + W
                        ]

                    for t in range(1, 8):
                        nc.tensor.matmul(
                            acc[:],
                            pw[:, (t - 1) * COUT : t * COUT],
                            rhs(w1, t),
                            start=(t == 1),
                            stop=False,
                        )
                    # box7_H(w2)[i] = w2o[i+1] - w2[i+8]
                    nc.tensor.matmul(acc[:], bw, rhs(w2o, 1), start=False, stop=False)
                    nc.tensor.matmul(acc[:], bwn, rhs(w2, 8), start=False, stop=True)

                    ot = opool.tile([COUT, OUT_TILE_FREE], BF16, tag="ot")
                    nc.scalar.activation(
                        ot[:], acc[:], AF.Identity, bias=bias_sb[:], scale=1.0
                    )
                    nc.sync.dma_start(
                        t_out[b, :, i0 : i0 + OUT_TILE_ROWS, :].rearrange(
                            "o r j -> o (r j)"
                        ),
                        ot[:],
                    )

    nc.compile()
    return nc


def make_in_maps(x, abc, bias):
    import ml_dtypes

    bf16 = ml_dtypes.bfloat16
    # host-padded row layout: 8 lead zeros + 112 data + 4 trail per row,
    # plus 8 guard zeros after the last row (slab reads overlap by 8)
    xpad = np.zeros((B_TOT, CIN, H, PW), dtype=bf16)
    xpad[:, :, :, LEAD : LEAD + W] = np.asarray(x)
    x = np.concatenate(
        [
            xpad.reshape(B_TOT, CIN, H * PW),
            np.zeros((B_TOT, CIN, 8), dtype=bf16),
        ],
        axis=2,
    )
    abc = np.asarray(abc, dtype=np.float32)
    bias = np.asarray(bias, dtype=np.float32)
    A, Bm, Cc = abc[0:128], abc[128:256], abc[256:384]
    pw = np.stack([(t - 4.0) * A + Cc for t in range(1, 8)]).astype(bf16)
    bw = np.stack([Bm, -Bm]).astype(bf16)
    in_maps = []
    for c in range(N_CORES):
        in_maps.append(
            {
                "xs": x[c * B_PER : (c + 1) * B_PER],
                "pw": np.ascontiguousarray(pw),
                "bw": np.ascontiguousarray(bw),
                "bias": np.ascontiguousarray(bias.reshape(COUT, 1)),
            }
        )
    return in_maps, N_CORES


def kernel(x: np.ndarray, abc: np.ndarray, bias: np.ndarray) -> np.ndarray:
    if "nc" not in _CACHE:
        _CACHE["nc"] = _build()
    nc = _CACHE["nc"]

    in_maps, _ = make_in_maps(x, abc, bias)
    res = run_bass_kernel_spmd(nc, in_maps, list(range(N_CORES)))
    out = np.concatenate(
        [np.asarray(res.results[c]["out"]) for c in range(N_CORES)], axis=0
    )
    return out.astype(np.float32)


if __name__ == "__main__":
    rng = np.random.default_rng(0)
    x = rng.standard_normal((16, 128, 112, 112), dtype=np.float32)
    abc = (rng.standard_normal((384, 128)) * 0.05).astype(np.float32)
    bias = (rng.standard_normal((128,)) * 0.05).astype(np.float32)
    out = kernel(x=x, abc=abc, bias=bias)
    print(out.shape, out.dtype)


# revision 28
# speedup vs baseline: 1.0281x; 1.0281x over previous
"""LConv (7x7 position-linear conv) Trainium2 Bass kernel.

Full inputs in, full output out. Sharding: data-parallel over batch,
16 images -> 8 NeuronCores (2 images/core). abc/bias replicated.

Math (K=7, S=1, P=3, D=1, G=1, C=COUT=128):
  out[o,i,j] = sum_c sum_{t=1..7} P_t[c,o]*W1[c,i+t-4,j]
             + sum_c B[c,o]*box7_H(w2)[c,i,j] + bias[o]
  W1 = 7-tap box along W of padded x; w2 = centered 7-tap ramp along W.
  P_t = (t-4)*A + C ; A=abc[0:128], B=abc[128:256], C=abc[256:384].

Rows are stored 124 wide (8 lead zeros + 112 data + 4 trail) so stream
cumsums telescope exactly:
  W1[k] = (S[k+8]-S[k]) - x[k+8]        BOXW7X2 - hand 2x-packed bf16 op
  w2[k] = 7*S0 + 4*(S1-S0) - cumsum(S1-S0), S1 = cumsum(x[+7])
                                        RAMPW7 - hand 1x op
box7_H(w2) is folded into the PE accumulation via pair/quad/oct row
sums (stock bf16 tensor_tensor at 2x, emitted in per-slab chunks so the
B-taps unlock progressively): box7 = oct[i+1] - w2[i+8], so each output
tile is 9 matmuls (7 w1-taps with the merged P_t weights + 2 w2-taps,
the second against a host-negated -B weight).
"""

import numpy as np

import concourse.bacc as bacc
import concourse.mybir as mybir
from concourse import tile
from concourse.bass_utils import run_bass_kernel_spmd

F32 = mybir.dt.float32
BF16 = mybir.dt.bfloat16
AF = mybir.ActivationFunctionType
ALU = mybir.AluOpType

B_TOT = 16
N_CORES = 8
B_PER = B_TOT // N_CORES
CIN = 128
COUT = 128
H = W = 112

LEAD = 8                  # row layout: 8 lead zeros + 112 data + 4 trail
TRAIL = 4
PW = LEAD + W + TRAIL     # 124 (even -> rows stay 4B-aligned in bf16)
COL0 = 4                  # rhs data col j lives at stream col j+4
GROWS = 119               # grid rows: 3 zeros + 112 data + 4 zeros
RTOP = 3                  # grid row of x-row 0
GFULL = GROWS * PW        # 14756

ROWS_PER_SLAB = 16
N_SLABS = H // ROWS_PER_SLAB
SLAB_FREE = ROWS_PER_SLAB * PW         # 1984
XP_FREE = SLAB_FREE + 8                # 8 guard zeros for the offset reads

OUT_TILE_ROWS = 4
N_OUT_TILES = H // OUT_TILE_ROWS
OUT_TILE_FREE = OUT_TILE_ROWS * W      # 448

_CACHE = {}


class _HandDveOp:
    """Duck-typed DveOp whose uop program is hand-authored (the Spec DSL
    cannot express scan-of-scan or packed 2x programs)."""

    def __init__(self, name, spec, subdim, build):
        self.name = name
        self.spec = spec
        self.subdim = subdim
        self._build = build
        self._cache = {}

    def compile(self, ver):
        if ver not in self._cache:
            self._cache[ver] = self._build(self.name, ver)
        return self._cache[ver]


def _register(name, build, reference):
    from concourse.dve_spec import Spec, Src0, Src1, scan, AluOp
    import concourse.dve_ops as dve_ops

    for op in dve_ops.OPS:
        if op.name == name:
            return op
    spec = Spec(
        body=scan(AluOp.ADD, Src0) - scan(AluOp.ADD, Src1),
        reference=reference,
    )
    row = dve_ops._CUSTOM_DVE_ROW_BASE + len(dve_ops.OPS)
    op = _HandDveOp(name, spec, False, build)
    dve_ops.OPS.append(op)
    dve_ops._SUB_OPCODE_FOR_NAME[name] = row
    dve_ops.CUSTOM_DVE_SPECS[name] = spec
    return op


def _mk_uop(UopConfig, wires):
    u = UopConfig()
    for sel, slot in wires:
        u.enable_input(sel, slot)
    return u


def _build_rampw7_uops(name, ver):
    """out = 7*S0 + 4*D - E ; S0=scan(src0), S1=scan(src1=x[+7]),
    D=S1-S0, E=scan(D). == sum_{t=1..7} (t-4)*x[k+t] given 7+ lead zeros."""
    from concourse.dve_uop import (
        DveOpSpec, UopConfig, UopDpConfig, AluOp, AluInp, InpSel, OutSel,
        OutPath, DelayInp, Trigger,
    )
    import concourse.dve_ops as dve_ops

    PREV, CURR = AluInp.PREV_ALU_OUT, AluInp.CURR_ALU_OUT

    def lane(n):
        return AluInp(int(AluInp.PREV_DELAY_0) + n)

    wires = [
        (InpSel.SRC_0, 1), (InpSel.SRC_1, 2),
        (InpSel.CONST_0, 3), (InpSel.CONST_1, 4), (InpSel.ZERO, 5),
    ]

    def body_dp():
        dp = [UopDpConfig() for _ in range(8)]
        for st in range(8):
            dp[st].pass_through_delay(0, 1, 2, 3, 4)
        dp[0].enable_alu(AluOp.ADD, CURR, lane(0))
        dp[1].enable_alu(AluOp.ADD, CURR, lane(1))
        dp[1].enable_delay_from_src(DelayInp.PREV_ALU_OUT, 0)
        dp[2].enable_alu(AluOp.SUBTRACT, PREV, lane(0))
        dp[3].enable_alu(AluOp.ADD, CURR, PREV)
        dp[3].enable_delay_from_src(DelayInp.PREV_ALU_OUT, 1)
        dp[4].enable_alu(AluOp.MULTIPLY, lane(0), lane(2))
        dp[4].enable_delay_from_src(DelayInp.PREV_ALU_OUT, 2)
        dp[5].enable_alu(AluOp.MULTIPLY, lane(1), lane(3))
        dp[5].enable_delay_from_src(DelayInp.PREV_ALU_OUT, 3)
        dp[6].enable_alu(AluOp.ADD, PREV, lane(3))
        dp[7].enable_alu(AluOp.SUBTRACT, PREV, lane(2))
        return dp

    seed = _mk_uop(UopConfig, wires)
    seed.datapath_config = body_dp()
    for b in (0, 1, 3):
        seed.datapath_config[b].enable_alu(AluOp.BYPASS, lane(4), lane(4))
    seed.trigger = (Trigger.COUNT, Trigger.NONE, Trigger.NONE)
    seed.repeat_count = 1
    seed.next_uop = (1, 0, 0)

    steady = _mk_uop(UopConfig, wires)
    steady.datapath_config = body_dp()
    steady.trigger = (Trigger.SRC_TENSOR_DONE, Trigger.NONE, Trigger.NONE)
    steady.require_inp0 = 1
    steady.require_inp1 = 1
    steady.enable_output(OutSel.ALU_OUT, OutPath.WR0_LO)

    spec = DveOpSpec(
        name=name, opcode=dve_ops.get_dve_sub_opcode(name),
        uops=[seed, steady], rd1_en=True,
    )
    spec.validate(ver)
    return spec


def _build_boxw7_uops(name, ver):
    """out[k] = sum_{t=1..7} x[k+t] = (S[k+8]-S[k]) - x[k+8], with
    src1 = x[+8] (16B offset keeps bf16 2x mode alignable)."""
    from concourse.dve_uop import (
        DveOpSpec, UopConfig, UopDpConfig, AluOp, AluInp, InpSel, OutSel,
        OutPath, DelayInp, Trigger,
    )
    import concourse.dve_ops as dve_ops

    PREV, CURR = AluInp.PREV_ALU_OUT, AluInp.CURR_ALU_OUT

    def lane(n):
        return AluInp(int(AluInp.PREV_DELAY_0) + n)

    # ---- 1x fallback ----
    wires1 = [(InpSel.SRC_0, 1), (InpSel.SRC_1, 2), (InpSel.ZERO, 3)]

    def dp1():
        dp = [UopDpConfig() for _ in range(8)]
        for st in range(8):
            dp[st].pass_through_delay(0, 1, 2)
        dp[0].enable_alu(AluOp.ADD, CURR, lane(0))
        dp[1].enable_alu(AluOp.ADD, CURR, lane(1))
        dp[1].enable_delay_from_src(DelayInp.PREV_ALU_OUT, 0)
        dp[2].enable_alu(AluOp.SUBTRACT, PREV, lane(0))
        dp[3].enable_alu(AluOp.SUBTRACT, PREV, lane(1))
        for st in range(4, 8):
            dp[st].pass_through_alu()
        return dp

    seed1 = _mk_uop(UopConfig, wires1)
    seed1.datapath_config = dp1()
    for b in (0, 1):
        seed1.datapath_config[b].enable_alu(AluOp.BYPASS, lane(2), lane(2))
    seed1.trigger = (Trigger.COUNT, Trigger.NONE, Trigger.NONE)
    seed1.repeat_count = 1
    seed1.next_uop = (1, 0, 0)
    steady1 = _mk_uop(UopConfig, wires1)
    steady1.datapath_config = dp1()
    steady1.trigger = (Trigger.SRC_TENSOR_DONE, Trigger.NONE, Trigger.NONE)
    steady1.require_inp0 = 1
    steady1.require_inp1 = 1
    steady1.enable_output(OutSel.ALU_OUT, OutPath.WR0_LO)

    # ---- 2x packed ----
    wires2 = [
        (InpSel.SRC_0, 1), (InpSel.SRC_0_HI, 2),
        (InpSel.SRC_1, 3), (InpSel.SRC_1_HI, 4), (InpSel.ZERO, 5),
    ]

    def dp2():
        dp = [UopDpConfig() for _ in range(8)]
        for st in range(8):
            dp[st].pass_through_delay(0, 1, 2, 3, 4)
        dp[0].enable_alu(AluOp.ADD, lane(0), lane(1))      # t0 = lo0+hi0
        dp[1].enable_alu(AluOp.ADD, CURR, PREV)            # sA = S[2m+1]
        dp[2].enable_alu(AluOp.ADD, lane(2), lane(3))      # t1 = lo1+hi1
        dp[2].enable_delay_from_src(DelayInp.PREV_ALU_OUT, 0)   # sA -> l0
        dp[3].enable_alu(AluOp.ADD, CURR, PREV)            # sB = S[2m+9]
        dp[4].enable_alu(AluOp.SUBTRACT, PREV, lane(0))    # Bo = sB-sA
        dp[5].enable_alu(AluOp.SUBTRACT, PREV, lane(3))    # W1o = Bo-hi1
        dp[6].enable_alu(AluOp.ADD, PREV, lane(1))         # u = W1o+hi0
        dp[6].enable_delay_from_src(DelayInp.PREV_ALU_OUT, 3)   # W1o -> l3
        dp[7].enable_alu(AluOp.SUBTRACT, PREV, lane(2))    # W1e = u-lo1
        return dp

    seed2 = _mk_uop(UopConfig, wires2)
    seed2.datapath_config = dp2()
    for b in (1, 3):
        seed2.datapath_config[b].enable_alu(AluOp.BYPASS, lane(4), lane(4))
    seed2.trigger = (Trigger.COUNT, Trigger.NONE, Trigger.NONE)
    seed2.repeat_count = 1
    seed2.next_uop = (1, 0, 0)
    steady2 = _mk_uop(UopConfig, wires2)
    steady2.datapath_config = dp2()
    steady2.trigger = (Trigger.SRC_TENSOR_DONE, Trigger.NONE, Trigger.NONE)
    steady2.require_inp0 = 1
    steady2.require_inp1 = 1
    steady2.enable_output(OutSel.ALU_OUT, OutPath.WR0_LO)
    steady2.enable_output(OutSel.DELAY_3, OutPath.WR0_HI)

    spec = DveOpSpec(
        name=name, opcode=dve_ops.get_dve_sub_opcode(name),
        uops=[seed1, steady1], uops_2x=[seed2, steady2],
        perf_max=1, rd1_en=True,
    )
    spec.validate(ver)
    return spec


def _ramp_ref(in0, in1, *a):
    s0 = np.cumsum(in0, axis=-1, dtype=np.float32)
    s1 = np.cumsum(in1, axis=-1, dtype=np.float32)
    d = s1 - s0
    return 7.0 * s0 + 4.0 * d - np.cumsum(d, axis=-1, dtype=np.float32)


def _box_ref(in0, in1, *a):
    s0 = np.cumsum(in0, axis=-1, dtype=np.float32)
    s1 = np.cumsum(in1, axis=-1, dtype=np.float32)
    return (s1 - s0) - in1


def _build():
    nc = bacc.Bacc("TRN2", target_bir_lowering=False, debug=False)
    op_box = _register("BOXW7X2", _build_boxw7_uops, _box_ref)
    op_ramp = _register("RAMPW7", _build_rampw7_uops, _ramp_ref)

    t_x = nc.dram_tensor(
        "xs", [B_PER, CIN, H * PW + 8], BF16, kind="ExternalInput"
    )
    t_pw = nc.dram_tensor("pw", [7, CIN, COUT], BF16, kind="ExternalInput")
    t_bw = nc.dram_tensor("bw", [2, CIN, COUT], BF16, kind="ExternalInput")
    t_bias = nc.dram_tensor("bias", [COUT, 1], F32, kind="ExternalInput")
    t_out = nc.dram_tensor("out", [B_PER, COUT, H, W], BF16, kind="ExternalOutput")

    with tile.TileContext(nc) as tc:
        with (
            tc.tile_pool(name="const", bufs=1) as cpool,
            tc.tile_pool(name="wfull", bufs=1) as wpool,
            tc.tile_pool(name="slab", bufs=2) as spool,
            tc.tile_pool(name="outs", bufs=4) as opool,
            tc.tile_pool(name="ps", bufs=8, space="PSUM") as ppool,
        ):
            # first slab's input DMA goes ahead of the const DMAs so the
            # DVE filter chain (and with it the first matmul) starts ~4us
            # earlier; the Sync queue dispatches descriptors in order.
            xp_bufs = []
            for i in range(2):
                xpb = spool.tile([CIN, XP_FREE], BF16, tag=f"xp{i}")
                xp_bufs.append(xpb)
            nc.sync.dma_start(xp_bufs[0][:], t_x[0, :, 0:XP_FREE])

            # ---- constants (host-prequantized bf16) ----
            pw = cpool.tile([CIN, 7 * COUT], BF16, tag="pwb")
            nc.sync.dma_start(
                pw[:].rearrange("c (t o) -> c t o", t=7),
                t_pw[:].transpose([1, 0, 2]),
            )
            bw2 = cpool.tile([CIN, 2 * COUT], BF16, tag="bwb")
            nc.sync.dma_start(
                bw2[:].rearrange("c (s o) -> c s o", s=2),
                t_bw[:].transpose([1, 0, 2]),
            )
            bw = bw2[:, :COUT]        # +B
            bwn = bw2[:, COUT:]       # -B
            bias_sb = cpool.tile([COUT, 1], F32, tag="bias")
            nc.sync.dma_start(bias_sb[:], t_bias[:])

            # ---- streams (all row-major [GROWS x PW] grids, bf16) ----
            w1 = wpool.tile([CIN, GFULL], BF16, tag="w1")
            w2 = wpool.tile([CIN, GFULL], BF16, tag="w2")
            w2p = wpool.tile([CIN, GFULL], BF16, tag="w2p")
            w2q = wpool.tile([CIN, GFULL], BF16, tag="w2q")
            w2o = wpool.tile([CIN, GFULL], BF16, tag="w2o")
            for buf in (w1, w2):
                nc.gpsimd.memset(buf[:, : RTOP * PW], 0.0)
                nc.gpsimd.memset(buf[:, (RTOP + H) * PW :], 0.0)

            # PE warm-up: the HAM clock gate starts at 1.2 GHz and needs
            # ~3.4us of sustained matmul activity to release to 2.4 GHz;
            # without this the first ~9 real matmuls run at half clock
            # while the DMA + filter chain fills the ~12us startup. Dummy
            # matmuls on the (already-memset, never-rewritten) w1 pad row
            # keep the PE busy; results land in rotating PSUM bufs that
            # are never read.
            for k in range(45):
                acc_d = ppool.tile([COUT, OUT_TILE_FREE], F32, tag="acc")
                nc.tensor.matmul(
                    acc_d[:, :128], w1[:, :128], w1[:, :128],
                    start=True, stop=True,
                )

            for b in range(B_PER):
                # ---------- stage 1: W-direction filters ----------
                for s in range(N_SLABS):
                    r0 = s * ROWS_PER_SLAB
                    xp = xp_bufs[s % 2]
                    if not (b == 0 and s == 0):
                        nc.sync.dma_start(
                            xp[:],
                            t_x[b, :, r0 * PW : r0 * PW + XP_FREE],
                        )
                    g0 = (RTOP + r0) * PW
                    inst = nc.vector._custom_dve(
                        op_box,
                        out=w1[:, g0 : g0 + SLAB_FREE],
                        in0=xp[:, :SLAB_FREE],
                        in1=xp[:, 8 : 8 + SLAB_FREE],
                    )
                    inst.ins.perf_max = 1

                    nc.vector._custom_dve(
                        op_ramp,
                        out=w2[:, g0 : g0 + SLAB_FREE],
                        in0=xp[:, :SLAB_FREE],
                        in1=xp[:, 7 : 7 + SLAB_FREE],
                        s0=7.0,
                        s1=4.0,
                    )

                    # H-direction pair/quad/oct rows, chunked per slab so
                    # the B-taps unblock progressively:
                    #   w2p[r] = w2[r] + w2[r+1]; w2q[r] = w2p[r] + w2p[r+2]
                    #   w2o[r] = w2q[r] + w2q[r+4]   (8-row sum)
                    def pq_chunk(eng, dst, src, lag, rlo, rhi):
                        if rhi < rlo:
                            return
                        a, bnd = rlo * PW, (rhi + 1) * PW
                        eng.tensor_tensor(
                            dst[:, a:bnd],
                            src[:, a:bnd],
                            src[:, a + lag * PW : bnd + lag * PW],
                            op=ALU.add,
                        )

                    vec = nc.vector
                    plo = 0 if s == 0 else 16 * s + 2
                    pq_chunk(vec, w2p, w2, 1, plo, 16 * s + 17)
                    qlo = 0 if s == 0 else 16 * s
                    pq_chunk(vec, w2q, w2p, 2, qlo, 16 * s + 15)
                    olo = 0 if s == 0 else 16 * s - 4
                    pq_chunk(vec, w2o, w2q, 4, olo, 16 * s + 11)
                    if s == N_SLABS - 1:
                        pq_chunk(vec, w2p, w2, 1, 16 * s + 18, GROWS - 2)
                        pq_chunk(vec, w2q, w2p, 2, 16 * s + 16, GROWS - 4)
                        pq_chunk(vec, w2o, w2q, 4, 16 * s + 12, GROWS - 8)

                # ---------- stage 2: PE taps ----------
                for it in range(N_OUT_TILES):
                    i0 = it * OUT_TILE_ROWS
                    acc = ppool.tile([COUT, OUT_TILE_FREE], F32, tag="acc")

                    def rhs(buf, trow):
                        # grid rows (i0+trow-1)..+3, data cols
                        base = (i0 + trow - 1) * PW
                        return buf[
                            :, base : base + OUT_TILE_ROWS * PW
                        ].rearrange("c (r q) -> c r q", q=PW)[
                            :, :, COL0 : COL0 + W
                        ]

                    for t in range(1, 8):
                        nc.tensor.matmul(
                            acc[:],
                            pw[:, (t - 1) * COUT : t * COUT],
                            rhs(w1, t),
                            start=(t == 1),
                            stop=False,
                        )
                    # box7_H(w2)[i] = w2o[i+1] - w2[i+8]
                    nc.tensor.matmul(acc[:], bw, rhs(w2o, 1), start=False, stop=False)
                    nc.tensor.matmul(acc[:], bwn, rhs(w2, 8), start=False, stop=True)

                    ot = opool.tile([COUT, OUT_TILE_FREE], BF16, tag="ot")
                    nc.scalar.activation(
                        ot[:], acc[:], AF.Identity, bias=bias_sb[:], scale=1.0
                    )
                    nc.sync.dma_start(
                        t_out[b, :, i0 : i0 + OUT_TILE_ROWS, :].rearrange(
                            "o r j -> o (r j)"
                        ),
                        ot[:],
                    )

    nc.compile()
    return nc


def make_in_maps(x, abc, bias):
    import ml_dtypes

    bf16 = ml_dtypes.bfloat16
    # host-padded row layout: 8 lead zeros + 112 data + 4 trail per row,
    # plus 8 guard zeros after the last row (slab reads overlap by 8)
    xpad = np.zeros((B_TOT, CIN, H, PW), dtype=bf16)
    xpad[:, :, :, LEAD : LEAD + W] = np.asarray(x)
    x = np.concatenate(
        [
            xpad.reshape(B_TOT, CIN, H * PW),
            np.zeros((B_TOT, CIN, 8), dtype=bf16),
        ],
        axis=2,
    )
    abc = np.asarray(abc, dtype=np.float32)
    bias = np.asarray(bias, dtype=np.float32)
    A, Bm, Cc = abc[0:128], abc[128:256], abc[256:384]
    pw = np.stack([(t - 4.0) * A + Cc for t in range(1, 8)]).astype(bf16)
    bw = np.stack([Bm, -Bm]).astype(bf16)
    in_maps = []
    for c in range(N_CORES):
        in_maps.append(
            {
                "xs": x[c * B_PER : (c + 1) * B_PER],
                "pw": np.ascontiguousarray(pw),
                "bw": np.ascontiguousarray(bw),
                "bias": np.ascontiguousarray(bias.reshape(COUT, 1)),
            }
        )
    return in_maps, N_CORES


def kernel(x: np.ndarray, abc: np.ndarray, bias: np.ndarray) -> np.ndarray:
    if "nc" not in _CACHE:
        _CACHE["nc"] = _build()
    nc = _CACHE["nc"]

    in_maps, _ = make_in_maps(x, abc, bias)
    res = run_bass_kernel_spmd(nc, in_maps, list(range(N_CORES)))
    out = np.concatenate(
        [np.asarray(res.results[c]["out"]) for c in range(N_CORES)], axis=0
    )
    return out.astype(np.float32)


if __name__ == "__main__":
    rng = np.random.default_rng(0)
    x = rng.standard_normal((16, 128, 112, 112), dtype=np.float32)
    abc = (rng.standard_normal((384, 128)) * 0.05).astype(np.float32)
    bias = (rng.standard_normal((128,)) * 0.05).astype(np.float32)
    out = kernel(x=x, abc=abc, bias=bias)
    print(out.shape, out.dtype)


# revision 31
# speedup vs baseline: 1.0304x; 1.0023x over previous
"""LConv (7x7 position-linear conv) Trainium2 Bass kernel.

Full inputs in, full output out. Sharding: data-parallel over batch,
16 images -> 8 NeuronCores (2 images/core). abc/bias replicated.

Math (K=7, S=1, P=3, D=1, G=1, C=COUT=128):
  out[o,i,j] = sum_c sum_{t=1..7} P_t[c,o]*W1[c,i+t-4,j]
             + sum_c B[c,o]*box7_H(w2)[c,i,j] + bias[o]
  W1 = 7-tap box along W of padded x; w2 = centered 7-tap ramp along W.
  P_t = (t-4)*A + C ; A=abc[0:128], B=abc[128:256], C=abc[256:384].

Rows are stored 124 wide (8 lead zeros + 112 data + 4 trail) so stream
cumsums telescope exactly:
  W1[k] = (S[k+8]-S[k]) - x[k+8]        BOXW7X2 - hand 2x-packed bf16 op
  w2[k] = 7*S0 + 4*(S1-S0) - cumsum(S1-S0), S1 = cumsum(x[+7])
                                        RAMPW7 - hand 1x op
box7_H(w2) is folded into the PE accumulation via pair/quad/oct row
sums (stock bf16 tensor_tensor at 2x, emitted in per-slab chunks so the
B-taps unlock progressively): box7 = oct[i+1] - w2[i+8], so each output
tile is 9 matmuls (7 w1-taps with the merged P_t weights + 2 w2-taps,
the second against a host-negated -B weight).
"""

import numpy as np

import concourse.bacc as bacc
import concourse.mybir as mybir
from concourse import tile
from concourse.bass_utils import run_bass_kernel_spmd

F32 = mybir.dt.float32
BF16 = mybir.dt.bfloat16
AF = mybir.ActivationFunctionType
ALU = mybir.AluOpType

B_TOT = 16
N_CORES = 8
B_PER = B_TOT // N_CORES
CIN = 128
COUT = 128
H = W = 112

LEAD = 8                  # row layout: 8 lead zeros + 112 data + 4 trail
TRAIL = 4
PW = LEAD + W + TRAIL     # 124 (even -> rows stay 4B-aligned in bf16)
COL0 = 4                  # rhs data col j lives at stream col j+4
GROWS = 119               # grid rows: 3 zeros + 112 data + 4 zeros
RTOP = 3                  # grid row of x-row 0
GFULL = GROWS * PW        # 14756

ROWS_PER_SLAB = 16
N_SLABS = H // ROWS_PER_SLAB
SLAB_FREE = ROWS_PER_SLAB * PW         # 1984
XP_FREE = SLAB_FREE + 8                # 8 guard zeros for the offset reads

OUT_TILE_ROWS = 4
N_OUT_TILES = H // OUT_TILE_ROWS
OUT_TILE_FREE = OUT_TILE_ROWS * W      # 448

_CACHE = {}


class _HandDveOp:
    """Duck-typed DveOp whose uop program is hand-authored (the Spec DSL
    cannot express scan-of-scan or packed 2x programs)."""

    def __init__(self, name, spec, subdim, build):
        self.name = name
        self.spec = spec
        self.subdim = subdim
        self._build = build
        self._cache = {}

    def compile(self, ver):
        if ver not in self._cache:
            self._cache[ver] = self._build(self.name, ver)
        return self._cache[ver]


def _register(name, build, reference):
    from concourse.dve_spec import Spec, Src0, Src1, scan, AluOp
    import concourse.dve_ops as dve_ops

    for op in dve_ops.OPS:
        if op.name == name:
            return op
    spec = Spec(
        body=scan(AluOp.ADD, Src0) - scan(AluOp.ADD, Src1),
        reference=reference,
    )
    row = dve_ops._CUSTOM_DVE_ROW_BASE + len(dve_ops.OPS)
    op = _HandDveOp(name, spec, False, build)
    dve_ops.OPS.append(op)
    dve_ops._SUB_OPCODE_FOR_NAME[name] = row
    dve_ops.CUSTOM_DVE_SPECS[name] = spec
    return op


def _mk_uop(UopConfig, wires):
    u = UopConfig()
    for sel, slot in wires:
        u.enable_input(sel, slot)
    return u


def _build_rampw7_uops(name, ver):
    """out = 7*S0 + 4*D - E ; S0=scan(src0), S1=scan(src1=x[+7]),
    D=S1-S0, E=scan(D). == sum_{t=1..7} (t-4)*x[k+t] given 7+ lead zeros."""
    from concourse.dve_uop import (
        DveOpSpec, UopConfig, UopDpConfig, AluOp, AluInp, InpSel, OutSel,
        OutPath, DelayInp, Trigger,
    )
    import concourse.dve_ops as dve_ops

    PREV, CURR = AluInp.PREV_ALU_OUT, AluInp.CURR_ALU_OUT

    def lane(n):
        return AluInp(int(AluInp.PREV_DELAY_0) + n)

    wires = [
        (InpSel.SRC_0, 1), (InpSel.SRC_1, 2),
        (InpSel.CONST_0, 3), (InpSel.CONST_1, 4), (InpSel.ZERO, 5),
    ]

    def body_dp():
        dp = [UopDpConfig() for _ in range(8)]
        for st in range(8):
            dp[st].pass_through_delay(0, 1, 2, 3, 4)
        dp[0].enable_alu(AluOp.ADD, CURR, lane(0))
        dp[1].enable_alu(AluOp.ADD, CURR, lane(1))
        dp[1].enable_delay_from_src(DelayInp.PREV_ALU_OUT, 0)
        dp[2].enable_alu(AluOp.SUBTRACT, PREV, lane(0))
        dp[3].enable_alu(AluOp.ADD, CURR, PREV)
        dp[3].enable_delay_from_src(DelayInp.PREV_ALU_OUT, 1)
        dp[4].enable_alu(AluOp.MULTIPLY, lane(0), lane(2))
        dp[4].enable_delay_from_src(DelayInp.PREV_ALU_OUT, 2)
        dp[5].enable_alu(AluOp.MULTIPLY, lane(1), lane(3))
        dp[5].enable_delay_from_src(DelayInp.PREV_ALU_OUT, 3)
        dp[6].enable_alu(AluOp.ADD, PREV, lane(3))
        dp[7].enable_alu(AluOp.SUBTRACT, PREV, lane(2))
        return dp

    seed = _mk_uop(UopConfig, wires)
    seed.datapath_config = body_dp()
    for b in (0, 1, 3):
        seed.datapath_config[b].enable_alu(AluOp.BYPASS, lane(4), lane(4))
    seed.trigger = (Trigger.COUNT, Trigger.NONE, Trigger.NONE)
    seed.repeat_count = 1
    seed.next_uop = (1, 0, 0)

    steady = _mk_uop(UopConfig, wires)
    steady.datapath_config = body_dp()
    steady.trigger = (Trigger.SRC_TENSOR_DONE, Trigger.NONE, Trigger.NONE)
    steady.require_inp0 = 1
    steady.require_inp1 = 1
    steady.enable_output(OutSel.ALU_OUT, OutPath.WR0_LO)

    spec = DveOpSpec(
        name=name, opcode=dve_ops.get_dve_sub_opcode(name),
        uops=[seed, steady], rd1_en=True,
    )
    spec.validate(ver)
    return spec


def _build_boxw7_uops(name, ver):
    """out[k] = sum_{t=1..7} x[k+t] = (S[k+8]-S[k]) - x[k+8], with
    src1 = x[+8] (16B offset keeps bf16 2x mode alignable)."""
    from concourse.dve_uop import (
        DveOpSpec, UopConfig, UopDpConfig, AluOp, AluInp, InpSel, OutSel,
        OutPath, DelayInp, Trigger,
    )
    import concourse.dve_ops as dve_ops

    PREV, CURR = AluInp.PREV_ALU_OUT, AluInp.CURR_ALU_OUT

    def lane(n):
        return AluInp(int(AluInp.PREV_DELAY_0) + n)

    # ---- 1x fallback ----
    wires1 = [(InpSel.SRC_0, 1), (InpSel.SRC_1, 2), (InpSel.ZERO, 3)]

    def dp1():
        dp = [UopDpConfig() for _ in range(8)]
        for st in range(8):
            dp[st].pass_through_delay(0, 1, 2)
        dp[0].enable_alu(AluOp.ADD, CURR, lane(0))
        dp[1].enable_alu(AluOp.ADD, CURR, lane(1))
        dp[1].enable_delay_from_src(DelayInp.PREV_ALU_OUT, 0)
        dp[2].enable_alu(AluOp.SUBTRACT, PREV, lane(0))
        dp[3].enable_alu(AluOp.SUBTRACT, PREV, lane(1))
        for st in range(4, 8):
            dp[st].pass_through_alu()
        return dp

    seed1 = _mk_uop(UopConfig, wires1)
    seed1.datapath_config = dp1()
    for b in (0, 1):
        seed1.datapath_config[b].enable_alu(AluOp.BYPASS, lane(2), lane(2))
    seed1.trigger = (Trigger.COUNT, Trigger.NONE, Trigger.NONE)
    seed1.repeat_count = 1
    seed1.next_uop = (1, 0, 0)
    steady1 = _mk_uop(UopConfig, wires1)
    steady1.datapath_config = dp1()
    steady1.trigger = (Trigger.SRC_TENSOR_DONE, Trigger.NONE, Trigger.NONE)
    steady1.require_inp0 = 1
    steady1.require_inp1 = 1
    steady1.enable_output(OutSel.ALU_OUT, OutPath.WR0_LO)

    # ---- 2x packed ----
    wires2 = [
        (InpSel.SRC_0, 1), (InpSel.SRC_0_HI, 2),
        (InpSel.SRC_1, 3), (InpSel.SRC_1_HI, 4), (InpSel.ZERO, 5),
    ]

    def dp2():
        dp = [UopDpConfig() for _ in range(8)]
        for st in range(8):
            dp[st].pass_through_delay(0, 1, 2, 3, 4)
        dp[0].enable_alu(AluOp.ADD, lane(0), lane(1))      # t0 = lo0+hi0
        dp[1].enable_alu(AluOp.ADD, CURR, PREV)            # sA = S[2m+1]
        dp[2].enable_alu(AluOp.ADD, lane(2), lane(3))      # t1 = lo1+hi1
        dp[2].enable_delay_from_src(DelayInp.PREV_ALU_OUT, 0)   # sA -> l0
        dp[3].enable_alu(AluOp.ADD, CURR, PREV)            # sB = S[2m+9]
        dp[4].enable_alu(AluOp.SUBTRACT, PREV, lane(0))    # Bo = sB-sA
        dp[5].enable_alu(AluOp.SUBTRACT, PREV, lane(3))    # W1o = Bo-hi1
        dp[6].enable_alu(AluOp.ADD, PREV, lane(1))         # u = W1o+hi0
        dp[6].enable_delay_from_src(DelayInp.PREV_ALU_OUT, 3)   # W1o -> l3
        dp[7].enable_alu(AluOp.SUBTRACT, PREV, lane(2))    # W1e = u-lo1
        return dp

    seed2 = _mk_uop(UopConfig, wires2)
    seed2.datapath_config = dp2()
    for b in (1, 3):
        seed2.datapath_config[b].enable_alu(AluOp.BYPASS, lane(4), lane(4))
    seed2.trigger = (Trigger.COUNT, Trigger.NONE, Trigger.NONE)
    seed2.repeat_count = 1
    seed2.next_uop = (1, 0, 0)
    steady2 = _mk_uop(UopConfig, wires2)
    steady2.datapath_config = dp2()
    steady2.trigger = (Trigger.SRC_TENSOR_DONE, Trigger.NONE, Trigger.NONE)
    steady2.require_inp0 = 1
    steady2.require_inp1 = 1
    steady2.enable_output(OutSel.ALU_OUT, OutPath.WR0_LO)
    steady2.enable_output(OutSel.DELAY_3, OutPath.WR0_HI)

    spec = DveOpSpec(
        name=name, opcode=dve_ops.get_dve_sub_opcode(name),
        uops=[seed1, steady1], uops_2x=[seed2, steady2],
        perf_max=1, rd1_en=True,
    )
    spec.validate(ver)
    return spec


def _ramp_ref(in0, in1, *a):
    s0 = np.cumsum(in0, axis=-1, dtype=np.float32)
    s1 = np.cumsum(in1, axis=-1, dtype=np.float32)
    d = s1 - s0
    return 7.0 * s0 + 4.0 * d - np.cumsum(d, axis=-1, dtype=np.float32)


def _box_ref(in0, in1, *a):
    s0 = np.cumsum(in0, axis=-1, dtype=np.float32)
    s1 = np.cumsum(in1, axis=-1, dtype=np.float32)
    return (s1 - s0) - in1


def _build():
    nc = bacc.Bacc("TRN2", target_bir_lowering=False, debug=False)
    op_box = _register("BOXW7X2", _build_boxw7_uops, _box_ref)
    op_ramp = _register("RAMPW7", _build_rampw7_uops, _ramp_ref)

    t_x = nc.dram_tensor(
        "xs", [B_PER, CIN, H * PW + 8], BF16, kind="ExternalInput"
    )
    t_pw = nc.dram_tensor("pw", [7, CIN, COUT], BF16, kind="ExternalInput")
    t_bw = nc.dram_tensor("bw", [2, CIN, COUT], BF16, kind="ExternalInput")
    t_bias = nc.dram_tensor("bias", [COUT, 1], F32, kind="ExternalInput")
    t_out = nc.dram_tensor("out", [B_PER, COUT, H, W], BF16, kind="ExternalOutput")

    with tile.TileContext(nc) as tc:
        with (
            tc.tile_pool(name="const", bufs=1) as cpool,
            tc.tile_pool(name="wfull", bufs=1) as wpool,
            tc.tile_pool(name="slab", bufs=2) as spool,
            tc.tile_pool(name="outs", bufs=4) as opool,
            tc.tile_pool(name="ps", bufs=8, space="PSUM") as ppool,
        ):
            # first slab's input DMA goes ahead of the const DMAs so the
            # DVE filter chain (and with it the first matmul) starts ~4us
            # earlier; the Sync queue dispatches descriptors in order.
            xp_bufs = []
            for i in range(2):
                xpb = spool.tile([CIN, XP_FREE], BF16, tag=f"xp{i}")
                xp_bufs.append(xpb)
            # ...and in two halves, so the first filter op (and with it the
            # first real matmul) is gated by only half a slab's transfer
            HALF = SLAB_FREE // 2
            nc.sync.dma_start(xp_bufs[0][:, : HALF + 8], t_x[0, :, 0 : HALF + 8])
            nc.sync.dma_start(
                xp_bufs[0][:, HALF + 8 :], t_x[0, :, HALF + 8 : XP_FREE]
            )

            # ---- constants (host-prequantized bf16) ----
            pw = cpool.tile([CIN, 7 * COUT], BF16, tag="pwb")
            nc.sync.dma_start(
                pw[:].rearrange("c (t o) -> c t o", t=7),
                t_pw[:].transpose([1, 0, 2]),
            )
            bw2 = cpool.tile([CIN, 2 * COUT], BF16, tag="bwb")
            nc.sync.dma_start(
                bw2[:].rearrange("c (s o) -> c s o", s=2),
                t_bw[:].transpose([1, 0, 2]),
            )
            bw = bw2[:, :COUT]        # +B
            bwn = bw2[:, COUT:]       # -B
            bias_sb = cpool.tile([COUT, 1], F32, tag="bias")
            nc.sync.dma_start(bias_sb[:], t_bias[:])

            # ---- streams (all row-major [GROWS x PW] grids, bf16) ----
            w1 = wpool.tile([CIN, GFULL], BF16, tag="w1")
            w2 = wpool.tile([CIN, GFULL], BF16, tag="w2")
            w2p = wpool.tile([CIN, GFULL], BF16, tag="w2p")
            w2q = wpool.tile([CIN, GFULL], BF16, tag="w2q")
            w2o = wpool.tile([CIN, GFULL], BF16, tag="w2o")
            for buf in (w1, w2):
                nc.gpsimd.memset(buf[:, : RTOP * PW], 0.0)
                nc.gpsimd.memset(buf[:, (RTOP + H) * PW :], 0.0)

            # PE warm-up: the HAM clock gate starts at 1.2 GHz and needs
            # ~3.4us of sustained matmul activity to release to 2.4 GHz;
            # without this the first ~9 real matmuls run at half clock
            # while the DMA + filter chain fills the ~12us startup. Dummy
            # matmuls on the (already-memset, never-rewritten) w1 pad row
            # keep the PE busy; results land in rotating PSUM bufs that
            # are never read.
            for k in range(28):
                acc_d = ppool.tile([COUT, OUT_TILE_FREE], F32, tag="acc")
                nc.tensor.matmul(
                    acc_d[:, :128], w1[:, :128], w1[:, :128],
                    start=True, stop=True,
                )

            for b in range(B_PER):
                # ---------- stage 1: W-direction filters ----------
                for s in range(N_SLABS):
                    r0 = s * ROWS_PER_SLAB
                    xp = xp_bufs[s % 2]
                    if not (b == 0 and s == 0):
                        nc.sync.dma_start(
                            xp[:],
                            t_x[b, :, r0 * PW : r0 * PW + XP_FREE],
                        )
                    g0 = (RTOP + r0) * PW
                    # first slab of the first image runs as two 8-row halves
                    # (matching its split DMA) so the leading matmuls unlock
                    # earlier; each half-stream starts at a row boundary, so
                    # the 8 lead zeros reset the telescoping scans.
                    halves = (
                        [(0, SLAB_FREE // 2), (SLAB_FREE // 2, SLAB_FREE)]
                        if (b == 0 and s == 0)
                        else [(0, SLAB_FREE)]
                    )
                    for h0, h1 in halves:
                        inst = nc.vector._custom_dve(
                            op_box,
                            out=w1[:, g0 + h0 : g0 + h1],
                            in0=xp[:, h0:h1],
                            in1=xp[:, h0 + 8 : h1 + 8],
                        )
                        inst.ins.perf_max = 1
                        nc.vector._custom_dve(
                            op_ramp,
                            out=w2[:, g0 + h0 : g0 + h1],
                            in0=xp[:, h0:h1],
                            in1=xp[:, h0 + 7 : h1 + 7],
                            s0=7.0,
                            s1=4.0,
                        )

                    # H-direction pair/quad/oct rows, chunked per slab so
                    # the B-taps unblock progressively:
                    #   w2p[r] = w2[r] + w2[r+1]; w2q[r] = w2p[r] + w2p[r+2]
                    #   w2o[r] = w2q[r] + w2q[r+4]   (8-row sum)
                    def pq_chunk(eng, dst, src, lag, rlo, rhi):
                        if rhi < rlo:
                            return
                        a, bnd = rlo * PW, (rhi + 1) * PW
                        eng.tensor_tensor(
                            dst[:, a:bnd],
                            src[:, a:bnd],
                            src[:, a + lag * PW : bnd + lag * PW],
                            op=ALU.add,
                        )

                    vec = nc.vector
                    plo = 0 if s == 0 else 16 * s + 2
                    pq_chunk(vec, w2p, w2, 1, plo, 16 * s + 17)
                    qlo = 0 if s == 0 else 16 * s
                    pq_chunk(vec, w2q, w2p, 2, qlo, 16 * s + 15)
                    olo = 0 if s == 0 else 16 * s - 4
                    pq_chunk(vec, w2o, w2q, 4, olo, 16 * s + 11)
                    if s == N_SLABS - 1:
                        pq_chunk(vec, w2p, w2, 1, 16 * s + 18, GROWS - 2)
                        pq_chunk(vec, w2q, w2p, 2, 16 * s + 16, GROWS - 4)
                        pq_chunk(vec, w2o, w2q, 4, 16 * s + 12, GROWS - 8)

                # ---------- stage 2: PE taps ----------
                for it in range(N_OUT_TILES):
                    i0 = it * OUT_TILE_ROWS
                    acc = ppool.tile([COUT, OUT_TILE_FREE], F32, tag="acc")

                    def rhs(buf, trow):
                        # grid rows (i0+trow-1)..+3, data cols
                        base = (i0 + trow - 1) * PW
                        return buf[
                            :, base : base + OUT_TILE_ROWS * PW
                        ].rearrange("c (r q) -> c r q", q=PW)[
                            :, :, COL0 : COL0 + W
                        ]

                    for t in range(1, 8):
                        nc.tensor.matmul(
                            acc[:],
                            pw[:, (t - 1) * COUT : t * COUT],
                            rhs(w1, t),
                            start=(t == 1),
                            stop=False,
                        )
                    # box7_H(w2)[i] = w2o[i+1] - w2[i+8]
                    nc.tensor.matmul(acc[:], bw, rhs(w2o, 1), start=False, stop=False)
                    nc.tensor.matmul(acc[:], bwn, rhs(w2, 8), start=False, stop=True)

                    ot = opool.tile([COUT, OUT_TILE_FREE], BF16, tag="ot")
                    nc.scalar.activation(
                        ot[:], acc[:], AF.Identity, bias=bias_sb[:], scale=1.0
                    )
                    nc.sync.dma_start(
                        t_out[b, :, i0 : i0 + OUT_TILE_ROWS, :].rearrange(
                            "o r j -> o (r j)"
                        ),
                        ot[:],
                    )

    nc.compile()
    return nc


def make_in_maps(x, abc, bias):
    import ml_dtypes

    bf16 = ml_dtypes.bfloat16
    # host-padded row layout: 8 lead zeros + 112 data + 4 trail per row,
    # plus 8 guard zeros after the last row (slab reads overlap by 8)
    xpad = np.zeros((B_TOT, CIN, H, PW), dtype=bf16)
    xpad[:, :, :, LEAD : LEAD + W] = np.asarray(x)
    x = np.concatenate(
        [
            xpad.reshape(B_TOT, CIN, H * PW),
            np.zeros((B_TOT, CIN, 8), dtype=bf16),
        ],
        axis=2,
    )
    abc = np.asarray(abc, dtype=np.float32)
    bias = np.asarray(bias, dtype=np.float32)
    A, Bm, Cc = abc[0:128], abc[128:256], abc[256:384]
    pw = np.stack([(t - 4.0) * A + Cc for t in range(1, 8)]).astype(bf16)
    bw = np.stack([Bm, -Bm]).astype(bf16)
    in_maps = []
    for c in range(N_CORES):
        in_maps.append(
            {
                "xs": x[c * B_PER : (c + 1) * B_PER],
                "pw": np.ascontiguousarray(pw),
                "bw": np.ascontiguousarray(bw),
                "bias": np.ascontiguousarray(bias.reshape(COUT, 1)),
            }
        )
    return in_maps, N_CORES


def kernel(x: np.ndarray, abc: np.ndarray, bias: np.ndarray) -> np.ndarray:
    if "nc" not in _CACHE:
        _CACHE["nc"] = _build()
    nc = _CACHE["nc"]

    in_maps, _ = make_in_maps(x, abc, bias)
    res = run_bass_kernel_spmd(nc, in_maps, list(range(N_CORES)))
    out = np.concatenate(
        [np.asarray(res.results[c]["out"]) for c in range(N_CORES)], axis=0
    )
    return out.astype(np.float32)


if __name__ == "__main__":
    rng = np.random.default_rng(0)
    x = rng.standard_normal((16, 128, 112, 112), dtype=np.float32)
    abc = (rng.standard_normal((384, 128)) * 0.05).astype(np.float32)
    bias = (rng.standard_normal((128,)) * 0.05).astype(np.float32)
    out = kernel(x=x, abc=abc, bias=bias)
    print(out.shape, out.dtype)
